# revision 43
# baseline (speedup 1.0000x reference)
"""Causal self-attention (B=4, T=2048, C=768, H=12) on 8 trn2 NeuronCores.

Sharding: 8 cores = 4 batches x 2 head-groups (6 heads each).
Each core: QKV projection for its 6 heads, causal attention, partial output
projection (row-parallel). Host sums the two partials per batch + b_proj.

v2 dataflow (empirically grounded on trn2):
  - Matmuls with <128 contraction partitions stream at HALF rate (2cyc/col).
    Scores therefore use zero-padded per-head Q tiles [128, T] against
    2-head-packed K tiles so the contraction is a full 128 partitions.
  - QKV (q,k) and V (tokens >= 512) and AV (queries >= 512) use fp8e4
    DoubleRow matmuls (2 contraction values per partition -> 2x).
    Weights/x scaled x16 into fp8's normal range, unscaled in the
    PSUM->SBUF copies. First 512 tokens/queries keep a bf16 V/AV path
    (softmax over few elements does not average out fp8 noise).
  - Flash-style denominator: V blocks carry a ones column; AV matmul
    accumulates y^T and the denominator in one pass.
  - Output partials shipped f16 (halves the output DMA).
"""

import os
import sys
import types

sys.path.insert(0, "/opt/trn_rl_repo")

import ml_dtypes
import numpy as np

import concourse.bass as bass
import concourse.tile as tile
from concourse import bacc, mybir
from concourse.bass_utils import run_bass_kernel_spmd

B, T, C, H, D = 4, 2048, 768, 12, 64
N_CORES = 8
HPC = H // 2          # heads per core = 6
QC = T // 512         # 4 query chunks of 512
TT = T // 128         # 16 token tiles
S = 16.0              # fp8 weight prescale
F32 = mybir.dt.float32
F16 = mybir.dt.float16
BF16 = mybir.dt.bfloat16
FP8 = mybir.dt.float8e4
NPBF = ml_dtypes.bfloat16
NPF8 = ml_dtypes.float8_e4m3


def _install_ntff_hook():
    """The image's antenv lacks axon_hooks; inject it so trace=True works."""
    if "antenv.axon_hooks" in sys.modules:
        return
    try:
        import antenv
        mod = types.ModuleType("antenv.axon_hooks")
        _state = {"hook": None}
        mod.set_axon_ntff_profile_hook = lambda h: _state.__setitem__("hook", h)
        mod.get_axon_ntff_profile_hook = lambda: _state["hook"]
        sys.modules["antenv.axon_hooks"] = mod
        antenv.axon_hooks = mod
        from trn_agent_boot.trn_boot import _ntff_profile_via_ctypes
        mod.set_axon_ntff_profile_hook(
            _ntff_profile_via_ctypes("/opt/axon/libaxon_pjrt.so")
        )
    except Exception:
        pass


def _build_program(has_bv: bool, debug: bool = False, n_dev: int = N_CORES):
    nc = bacc.Bacc(
        "TRN2",
        target_bir_lowering=False,
        debug=False,
        enable_asserts=False,
        num_devices=n_dev,
    )
    xp8 = nc.dram_tensor("xp8", [128, 6 * T], FP8, kind="ExternalInput").ap()
    xb16 = nc.dram_tensor("xb16", [128, 6 * 512], BF16, kind="ExternalInput").ap()
    wqk8 = nc.dram_tensor("wqk8", [128, 4608], FP8, kind="ExternalInput").ap()
    wv8 = nc.dram_tensor("wv8", [128, 2400], FP8, kind="ExternalInput").ap()
    wv16 = nc.dram_tensor("wv16", [128, 2340], BF16, kind="ExternalInput").ap()
    wp16 = nc.dram_tensor("wp16", [128, 2304], BF16, kind="ExternalInput").ap()
    bqk = nc.dram_tensor("bqk", [128, 6], F32, kind="ExternalInput").ap()
    qmask = nc.dram_tensor("qmask", [128, 2], F32, kind="ExternalInput").ap()
    mask = nc.dram_tensor("mask", [128, 1024], BF16, kind="ExternalInput").ap()
    bv = nc.dram_tensor("bv", [1, 390], BF16, kind="ExternalInput").ap()
    ones = nc.dram_tensor("ones", [1, 128], BF16, kind="ExternalInput").ap()
    yp = nc.dram_tensor("yp", [T, C], F16, kind="ExternalOutput").ap()
    dbg = None
    if debug:
        dbg = {
            "dbg_q0": nc.dram_tensor("dbg_q0", [128, T], BF16,
                                     kind="ExternalOutput").ap(),
            "dbg_k0": nc.dram_tensor("dbg_k0", [128, T], BF16,
                                     kind="ExternalOutput").ap(),
            "dbg_v": nc.dram_tensor("dbg_v", [128, TT * 768], FP8,
                                    kind="ExternalOutput").ap(),
            "dbg_vb16": nc.dram_tensor("dbg_vb16", [128, 4 * 768], BF16,
                                       kind="ExternalOutput").ap(),
            "dbg_z": nc.dram_tensor("dbg_z", [128, 1024 * 8], F32,
                                    kind="ExternalOutput").ap(),
            "dbg_yz": nc.dram_tensor("dbg_yz", [128, 512 * 8], F32,
                                     kind="ExternalOutput").ap(),
        }

    with tile.TileContext(nc) as tc:
        _body(tc, nc, has_bv, xp8, xb16, wqk8, wv8, wv16, wp16, bqk, qmask,
              mask, bv, ones, yp, dbg)

    nc.compile()
    return nc


def _body(tc, nc, has_bv, xp8, xb16, wqk8, wv8, wv16, wp16, bqk, qmask,
          mask, bv, ones, yp, dbg=None):
    from contextlib import ExitStack
    DR = mybir.MatmulPerfMode.DoubleRow
    ADD = mybir.AluOpType.add
    MUL = mybir.AluOpType.mult
    EXP = mybir.ActivationFunctionType.Exp

    with ExitStack() as es:
        persist = es.enter_context(tc.tile_pool(name="persist", bufs=1))
        ppair = es.enter_context(tc.tile_pool(name="ppair", bufs=2, space="PSUM"))
        pyz = es.enter_context(tc.tile_pool(name="pyz", bufs=2, space="PSUM"))
        paux = es.enter_context(tc.tile_pool(name="paux", bufs=1, space="PSUM"))
        zpool = es.enter_context(tc.tile_pool(name="zpool", bufs=3))
        z16pool = es.enter_context(tc.tile_pool(name="z16pool", bufs=3))
        ypool = es.enter_context(tc.tile_pool(name="ypool", bufs=2))
        opool = es.enter_context(tc.tile_pool(name="opool", bufs=3))
        spool = es.enter_context(tc.tile_pool(name="spool", bufs=2))

        # ---- persistent tiles
        xp8s = persist.tile([128, 6 * T], FP8, tag="xp8", name="xp8s")
        xb16s = persist.tile([128, 6 * 512], BF16, tag="xb16", name="xb16s")
        wqk8s = persist.tile([128, 4608], FP8, tag="wqk8", name="wqk8s")
        wv8s = persist.tile([128, 2400], FP8, tag="wv8", name="wv8s")
        wv16s = persist.tile([128, 2340], BF16, tag="wv16", name="wv16s")
        wp16s = persist.tile([128, 2304], BF16, tag="wp16", name="wp16s")
        bqks = persist.tile([128, 6], F32, tag="bqk", name="bqks")
        qmasks = persist.tile([128, 2], F32, tag="qmask", name="qmasks")
        masks = persist.tile([128, 1024], BF16, tag="mask", name="masks")
        bvs = persist.tile([1, 390], BF16, tag="bv", name="bvs")
        oness = persist.tile([1, 128], BF16, tag="ones", name="oness")
        vbuf8 = persist.tile([128, TT * 768], FP8, tag="vbuf8", name="vbuf8")
        vb16 = persist.tile([128, 4 * 768], BF16, tag="vb16", name="vb16")
        qt = [persist.tile([128, T], BF16, tag=f"qt{h}", name=f"qt{h}")
              for h in range(HPC)]
        kt = [persist.tile([128, T], BF16, tag=f"kt{r}", name=f"kt{r}")
              for r in range(3)]

        # ---- DMA priority order. wqk8 is ft-major so b_round(0,0) only
        # gates on ft blocks 0 and 3; x window 0 on its own queue.
        nc.sync.dma_start(xp8s[:, 0:3072], xp8[:, 0:3072])
        nc.gpsimd.dma_start(wqk8s[:, 0:768], wqk8[:, 0:768])
        nc.gpsimd.dma_start(wqk8s[:, 2304:3072], wqk8[:, 2304:3072])
        nc.scalar.dma_start(bqks[:], bqk[:])
        nc.scalar.dma_start(qmasks[:], qmask[:])
        nc.scalar.dma_start(masks[:], mask[:])
        nc.scalar.dma_start(bvs[:], bv[:])
        nc.scalar.dma_start(oness[:], ones[:])
        nc.sync.dma_start(xb16s[:, 0:1536], xb16[:, 0:1536])
        nc.scalar.dma_start(xb16s[:, 1536:3072], xb16[:, 1536:3072])
        nc.gpsimd.dma_start(wv16s[:], wv16[:])
        nc.gpsimd.dma_start(wqk8s[:, 768:1536], wqk8[:, 768:1536])
        nc.gpsimd.dma_start(wqk8s[:, 3072:3840], wqk8[:, 3072:3840])
        nc.gpsimd.dma_start(wqk8s[:, 1536:2304], wqk8[:, 1536:2304])
        nc.gpsimd.dma_start(wqk8s[:, 3840:4608], wqk8[:, 3840:4608])
        nc.gpsimd.dma_start(wv8s[:], wv8[:])
        nc.gpsimd.dma_start(wp16s[:], wp16[:])

        # vbuf8 layout: block tt at tt*768, head h at +h*128: cols 0:64 = v,
        # cols 64:128 all ones -> the AV matmul replicates the softmax
        # denominator into yz rows 64:128 (free partition-broadcast).
        vb_blocks = vbuf8[:].rearrange("p (b f) -> p b f", b=TT * 6)
        nc.gpsimd.memset(vb_blocks[:, :, 64:128], 1.0)
        vb16_blocks = vb16[:].rearrange("p (b f) -> p b f", b=4 * 6)
        nc.gpsimd.memset(vb16_blocks[:, :, 64:128], 1.0)
        # z8 ring bufs zeroed once: diag-pair mask muls read stale regions
        # (finite garbage is fine, first-use NaN bit patterns are not)
        for _ in range(6):
            zi = zpool.tile([128, 1024], FP8, tag="z8", name="z8init")
            nc.gpsimd.memset(zi[:], 0.0)

        # x^T stored q4-major: [128, q4(4), chunk(6), 512]; each a_chunk DMA
        # is one contiguous 3KB-per-partition slab
        xq = xp8s[:].rearrange("p (q c t) -> p q c t", q=4, c=6)

        def xpair(p, q4):
            # fp8 x^T chunk-pair view for one 512-token window: [128, 2, 512]
            return xq[:, q4, 2 * p:2 * p + 2, :]

        def wqk_pair(p, ft):
            # ft-major layout: [128, ft(6), p(3), slot(2), 128]
            base = ft * 768 + p * 256
            return wqk8s[:, base:base + 256].rearrange(
                "p (two f) -> p two f", two=2)

        def wv_pair(p):
            return wv8s[:, p * 800:(p + 1) * 800].rearrange(
                "p (two f) -> p two f", two=2)[:, :, 0:390]

        def vpair(j, h):
            return vbuf8[:, 2 * j * 768:(2 * j + 2) * 768].rearrange(
                "p (two f) -> p two f", two=2)[:, :, h * 128:(h + 1) * 128]

        def a_chunk(q4):
            nc.sync.dma_start(xp8s[:, q4 * 3072:(q4 + 1) * 3072],
                              xp8[:, q4 * 3072:(q4 + 1) * 3072])

        def b_round(q4, r):
            # q features (ft=r) and k features (ft=r+3) for heads 2r, 2r+1
            w = slice(q4 * 512, (q4 + 1) * 512)
            pt = paux.tile([128, 1024], F32, tag="aux", name="ptqk")
            for p in range(3):
                rhs = xpair(p, q4)
                nc.tensor.matmul(pt[:, 0:512], wqk_pair(p, r), rhs,
                                 start=(p == 0), stop=(p == 2), perf_mode=DR)
                nc.tensor.matmul(pt[:, 512:1024], wqk_pair(p, r + 3), rhs,
                                 start=(p == 0), stop=(p == 2), perf_mode=DR)
            # (psum + 16*bias) * rowmask/16 -> unscaled q with dead rows zeroed
            nc.vector.tensor_scalar(
                qt[2 * r][:, w], pt[:, 0:512], bqks[:, r:r + 1],
                qmasks[:, 0:1], ADD, MUL)
            nc.vector.tensor_scalar(
                qt[2 * r + 1][:, w], pt[:, 0:512], bqks[:, r:r + 1],
                qmasks[:, 1:2], ADD, MUL)
            nc.vector.tensor_scalar(
                kt[r][:, w], pt[:, 512:1024], bqks[:, r + 3:r + 4],
                1.0 / S, ADD, MUL)

        def c_block(q4, j):
            tt = 4 * q4 + j
            pva = paux.tile([128, 1024], F32, tag="aux", name="pv")
            pv = pva[:, 0:390]
            if tt < 4:
                for cc in range(6):
                    nc.tensor.matmul(
                        pv,
                        xb16s[:, cc * 512 + tt * 128:cc * 512 + (tt + 1) * 128],
                        wv16s[:, cc * 390:(cc + 1) * 390],
                        start=(cc == 0), stop=(cc == 5) and not has_bv,
                    )
            else:
                for p in range(3):
                    nc.tensor.matmul(
                        pv,
                        xq[:, q4, 2 * p:2 * p + 2, j * 128:(j + 1) * 128],
                        wv_pair(p),
                        start=(p == 0), stop=(p == 2) and not has_bv,
                        perf_mode=DR,
                    )
            return tt, pv

        def c_finish(tt, pv):
            ncols = 65 if has_bv else 64
            if has_bv:
                nc.tensor.matmul(pv, oness[:], bvs[:],
                                 start=False, stop=True)
            # unscale x16 -> vbuf8 (fp8), strided to 128-col head slots
            nc.vector.tensor_scalar_mul(
                vb_blocks[:, tt * 6:(tt + 1) * 6, 0:ncols],
                pv.rearrange("p (b f) -> p b f", b=6)[:, :, 0:ncols],
                1.0 / S)
            if tt < 4:
                nc.vector.tensor_scalar_mul(
                    vb16_blocks[:, tt * 6:(tt + 1) * 6, 0:ncols],
                    pv.rearrange("p (b f) -> p b f", b=6)[:, :, 0:ncols],
                    1.0 / S)

        def c_piece(q4, j):
            c_finish(*c_block(q4, j))

        def sc_pair(q4, h, j):
            """Scores + exp + masks for pair j of head h of query chunk q4."""
            w0 = q4 * 512
            use8 = q4 > 0
            kb0, kb1 = 2 * j, 2 * j + 1
            off0 = max(0, kb0 * 128 - w0)
            off1 = max(0, kb1 * 128 - w0)
            diag1 = kb1 * 128 >= w0
            kk = kt[h // 2]
            qq = qt[h]
            pt = ppair.tile([128, 1024], F32, tag="pair", name="ptsc")
            nc.tensor.matmul(
                pt[:, off0:512], kk[:, kb0 * 128:(kb0 + 1) * 128],
                qq[:, w0 + off0:w0 + 512], start=True, stop=True)
            nc.tensor.matmul(
                pt[:, 512 + off1:1024], kk[:, kb1 * 128:(kb1 + 1) * 128],
                qq[:, w0 + off1:w0 + 512], start=True, stop=True)
            if use8:
                zt = zpool.tile([128, 1024], FP8, tag="z8", name="z8")
            else:
                zt = z16pool.tile([128, 1024], BF16, tag="z16", name="z16")
            if not diag1:
                nc.scalar.activation(zt[:], pt[:], EXP, scale=0.125)
            else:
                # diag pairs always have off1 == off0 + 128
                nc.scalar.activation(zt[:, off0:512], pt[:, off0:512],
                                     EXP, scale=0.125)
                nc.scalar.activation(zt[:, 512 + off1:1024],
                                     pt[:, 512 + off1:1024],
                                     EXP, scale=0.125)
                # slot0 causal band: cols off0:off0+128, keep j >= p+off0
                nc.gpsimd.tensor_mul(
                    zt[:, off0:off0 + 128], zt[:, off0:off0 + 128],
                    masks[:, 512:640])
                if use8:
                    # slot1: AV-DR reads cols >= off0; zero the stale region
                    # AND apply the band in one mul over [off0, off1+128)
                    nc.gpsimd.tensor_mul(
                        zt[:, 512 + off0:512 + off1 + 128],
                        zt[:, 512 + off0:512 + off1 + 128],
                        masks[:, 384:640])
                else:
                    nc.gpsimd.tensor_mul(
                        zt[:, 512 + off1:512 + off1 + 128],
                        zt[:, 512 + off1:512 + off1 + 128],
                        masks[:, 512:640])
            return (q4, h, j, zt, off0, off1)

        def av_pair(st, yz):
            q4, h, j, zt, off0, off1 = st
            npairs = 2 * q4 + 2
            kb0, kb1 = 2 * j, 2 * j + 1
            if q4 > 0:
                nc.tensor.matmul(
                    yz[:, off0:512],
                    vpair(j, h),
                    zt[:].rearrange("p (two n) -> p two n", two=2)
                    [:, :, off0:512],
                    start=(j == 0), stop=(j == npairs - 1), perf_mode=DR)
            else:
                nc.tensor.matmul(
                    yz[:, off0:512],
                    vb16[:, kb0 * 768 + h * 128:kb0 * 768 + (h + 1) * 128],
                    zt[:, off0:512],
                    start=(j == 0), stop=False)
                nc.tensor.matmul(
                    yz[:, off1:512],
                    vb16[:, kb1 * 768 + h * 128:kb1 * 768 + (h + 1) * 128],
                    zt[:, 512 + off1:1024],
                    start=False, stop=(j == npairs - 1))

        def norm_head(h, yz, yts):
            # yz rows 64:128 all hold the denominator (ones-cols trick)
            dcp = spool.tile([64, 512], F32, tag="dcp", name="dcp")
            nc.vector.tensor_copy(dcp[:], yz[64:128, :])
            rc = spool.tile([64, 512], F32, tag="rc", name="rc")
            nc.vector.reciprocal_approx_fast(rc[:], dcp[:])
            nc.vector.tensor_mul(
                yts[h // 2][(h % 2) * 64:(h % 2) * 64 + 64, :],
                yz[0:64, :], rc[:])

        def make_proj_halves(pq4, pyts, qt_i, pool, tag):
            state = {}

            def half0():
                state["ot"] = opool.tile([128, C], F16, tag="ot", name="ot")
                state["pp"] = pool.tile([128, 1024], F32, tag=tag, name="pp")
                for hdc in range(3):
                    nc.tensor.matmul(
                        state["pp"][:, 0:384],
                        pyts[hdc][:, qt_i * 128:(qt_i + 1) * 128],
                        wp16s[:, hdc * 768:hdc * 768 + 384],
                        start=(hdc == 0), stop=(hdc == 2))
                nc.vector.tensor_copy(
                    state["ot"][:, 0:384], state["pp"][:, 0:384])

            def half1():
                for hdc in range(3):
                    nc.tensor.matmul(
                        state["pp"][:, 512:896],
                        pyts[hdc][:, qt_i * 128:(qt_i + 1) * 128],
                        wp16s[:, hdc * 768 + 384:hdc * 768 + 768],
                        start=(hdc == 0), stop=(hdc == 2))
                nc.vector.tensor_copy(
                    state["ot"][:, 384:768], state["pp"][:, 512:896])
                row = (pq4 * 4 + qt_i) * 128
                nc.sync.dma_start(yp[row:row + 128, :], state["ot"][:])

            return half0, half1

        # ---- braided pipeline: a flat per-pair stream with the AV matmul
        # lagging TWO pairs behind its scores (the PE queue is in-order, so
        # an AV waiting on exp/mask would otherwise block the whole stream);
        # QKV rounds / V blocks (next q4) and proj halves (prev q4)
        # interleave at head boundaries as filler PE work.
        from collections import deque
        b_round(0, 0)
        c_piece(0, 0)
        c_piece(0, 1)
        pending = None
        for q4 in range(QC):
            if q4 + 1 < QC:
                a_chunk(q4 + 1)
            yts = [ypool.tile([128, 512], BF16, tag=f"yt{i}", name=f"yt{i}")
                   for i in range(3)]
            npairs = 2 * q4 + 2
            pairs = [(h, j) for h in range(HPC) for j in range(npairs)]
            nextf = []
            if pending is not None:
                for qt_i in range(4):
                    h0, h1 = make_proj_halves(*pending, qt_i, paux, "aux")
                    nextf.append(h0)
                    nextf.append(h1)
            if q4 + 1 < QC:
                for jj in range(4):
                    nextf.append(lambda jj=jj: c_piece(q4 + 1, jj))
                for r in range(3):
                    nextf.append(lambda r=r: b_round(q4 + 1, r))
            fill_at = {}
            if q4 == 0:
                fill_at = {1: [lambda: c_piece(0, 2)],
                           2: [lambda: c_piece(0, 3)],
                           4: [lambda: b_round(0, 1)],
                           6: [lambda: b_round(0, 2)]}
                for i, f in enumerate(nextf):
                    fill_at.setdefault(8 if i < 3 else 10, []).append(f)
            else:
                for i, f in enumerate(nextf):
                    fill_at.setdefault((i % HPC) * npairs, []).append(f)
            pend = deque()
            yzs = {}

            def flush_one():
                st = pend.popleft()
                ph = st[1]
                av_pair(st, yzs[ph])
                if st[2] == npairs - 1:
                    norm_head(ph, yzs[ph], yts)

            for idx, (h, j) in enumerate(pairs):
                for f in fill_at.get(idx, []):
                    f()
                if h not in yzs:
                    yzs[h] = pyz.tile([128, 512], F32, tag="yz",
                                      name=f"yz{h}")
                pend.append(sc_pair(q4, h, j))
                if len(pend) > 2:
                    flush_one()
            while pend:
                flush_one()
            pending = (q4, yts)
        # tail: no pairs left to hide behind; alternate PSUM pools so the
        # four proj pieces overlap each other
        for qt_i in range(4):
            pool, tag = ((ppair, "pair") if qt_i % 2 else (paux, "aux"))
            h0, h1 = make_proj_halves(*pending, qt_i, pool, tag)
            h0()
            h1()
        if dbg is not None:
            nc.sync.dma_start(dbg["dbg_q0"][:], qt[0][:])
            nc.sync.dma_start(dbg["dbg_k0"][:], kt[0][:])
            nc.sync.dma_start(dbg["dbg_v"][:], vbuf8[:])
            nc.sync.dma_start(dbg["dbg_vb16"][:], vb16[:])


_PROGRAMS = {}


def _get_program(has_bv):
    if has_bv not in _PROGRAMS:
        _PROGRAMS[has_bv] = _build_program(has_bv)
    return _PROGRAMS[has_bv]


def _pack_rows(a, npair):
    """[rows, f] -> [128, npair, 2, f] -> [128, npair*2*f] paired row chunks."""
    rows, f = a.shape
    return np.ascontiguousarray(
        a.reshape(npair, 2, 128, f).transpose(2, 0, 1, 3).reshape(128, -1))


def _pad_wv(wv):
    out = np.zeros((C, HPC * 65), dtype=np.float32)
    for h in range(HPC):
        out[:, h * 65:h * 65 + D] = wv[:, h * D:(h + 1) * D]
    return out


def kernel(x, W_attn, b_attn, W_proj, b_proj):
    x = np.ascontiguousarray(x, dtype=np.float32)
    W_attn = np.ascontiguousarray(W_attn, dtype=np.float32)
    b_attn = np.ascontiguousarray(b_attn, dtype=np.float32)
    W_proj = np.ascontiguousarray(W_proj, dtype=np.float32)
    b_proj = np.ascontiguousarray(b_proj, dtype=np.float32)

    has_bv = bool(np.any(b_attn[2 * C:]))
    nc = _get_program(has_bv)

    # mask[p, c] = 1 iff c >= p + 512: slicing [:, 512:640] gives the
    # causal triu band; [:, 384:640] additionally zeroes 128 garbage cols
    ci = np.arange(1024)[None, :]
    pi = np.arange(128)[:, None]
    mask_const = (ci >= pi + 512).astype(np.float32).astype(NPBF)
    qmask_const = np.zeros((128, 2), np.float32)
    qmask_const[0:64, 0] = 1.0 / S
    qmask_const[64:128, 1] = 1.0 / S
    ones_const = np.ones((1, 128), dtype=NPBF)

    in_maps = []
    for core in range(N_CORES):
        b, g = core // 2, core % 2
        qcols = slice(384 * g, 384 * (g + 1))
        kcols = slice(768 + 384 * g, 768 + 384 * (g + 1))
        vcols = slice(1536 + 384 * g, 1536 + 384 * (g + 1))

        xt = np.ascontiguousarray(x[b].T)                      # [768, 2048]
        # q4-major: xp8[p, q4*3072 + cc*512 + t'] = x^T[cc*128+p, q4*512+t']
        xp8_np = xt.reshape(6, 128, 4, 512).transpose(1, 2, 0, 3) \
            .reshape(128, 6 * T)
        xb16_np = xt[:, 0:512].reshape(6, 128, 512).transpose(1, 0, 2) \
            .reshape(128, 6 * 512)

        wcat = np.concatenate([W_attn[:, qcols], W_attn[:, kcols]], axis=1)
        # ft-major: wqk8[pr, ft*768 + p*256 + i*128 + f]
        #         = (wcat*S)[(2p+i)*128 + pr, ft*128 + f]
        wqk8_np = (wcat * S).reshape(3, 2, 128, 6, 128) \
            .transpose(2, 3, 0, 1, 4).reshape(128, 4608)

        wvp = _pad_wv(W_attn[:, vcols]) * S                    # [768, 390]
        wv8_pairs = wvp.reshape(3, 2, 128, 390).transpose(2, 0, 1, 3)
        wv8_np = np.zeros((128, 3, 2, 400), np.float32)
        wv8_np[:, :, :, 0:390] = wv8_pairs
        wv8_np = wv8_np.reshape(128, 2400)
        wv16_np = wvp.reshape(6, 128, 390).transpose(1, 0, 2).reshape(128, -1)

        wp16_np = W_proj[384 * g:384 * (g + 1), :] \
            .reshape(3, 128, C).transpose(1, 0, 2).reshape(128, -1)

        bqk_np = np.concatenate([b_attn[qcols], b_attn[kcols]]) \
            .reshape(6, 128).T * S

        bv_np = np.zeros((1, 390), np.float32)
        bvg = b_attn[vcols]
        for h in range(HPC):
            bv_np[0, h * 65:h * 65 + D] = bvg[h * D:(h + 1) * D] * S
            bv_np[0, h * 65 + D] = S

        in_maps.append({
            "xp8": xp8_np.astype(NPF8),
            "xb16": xb16_np.astype(NPBF),
            "wqk8": wqk8_np.astype(NPF8),
            "wv8": wv8_np.astype(NPF8),
            "wv16": wv16_np.astype(NPBF),
            "wp16": np.ascontiguousarray(wp16_np).astype(NPBF),
            "bqk": np.ascontiguousarray(bqk_np, dtype=np.float32),
            "qmask": qmask_const,
            "mask": mask_const,
            "bv": bv_np.astype(NPBF),
            "ones": ones_const,
        })

    trace = bool(int(os.environ.get("KBENCH_TRACE", "0")))
    if trace:
        _install_ntff_hook()
    res = run_bass_kernel_spmd(
        nc, in_maps, list(range(N_CORES)), trace=trace,
    )
    kernel.last_exec_time_ns = res.exec_time_ns

    out = np.empty((B, T, C), dtype=np.float32)
    for b in range(B):
        out[b] = (res.results[2 * b]["yp"].astype(np.float32)
                  + res.results[2 * b + 1]["yp"].astype(np.float32)
                  + b_proj)
    return out


# revision 48
# speedup vs baseline: 1.0542x; 1.0542x over previous
"""Causal self-attention (B=4, T=2048, C=768, H=12) on 8 trn2 NeuronCores.

Sharding: 8 cores = 4 batches x 2 head-groups (6 heads each).
Each core: QKV projection for its 6 heads, causal attention, partial output
projection (row-parallel). Host sums the two partials per batch + b_proj.

v2 dataflow (empirically grounded on trn2):
  - Matmuls with <128 contraction partitions stream at HALF rate (2cyc/col).
    Scores therefore use zero-padded per-head Q tiles [128, T] against
    2-head-packed K tiles so the contraction is a full 128 partitions.
  - QKV (q,k) and V (tokens >= 512) and AV (queries >= 512) use fp8e4
    DoubleRow matmuls (2 contraction values per partition -> 2x).
    Weights/x scaled x16 into fp8's normal range, unscaled in the
    PSUM->SBUF copies. First 512 tokens/queries keep a bf16 V/AV path
    (softmax over few elements does not average out fp8 noise).
  - Flash-style denominator: V blocks carry a ones column; AV matmul
    accumulates y^T and the denominator in one pass.
  - Output partials shipped f16 (halves the output DMA).
"""

import os
import sys
import types

sys.path.insert(0, "/opt/trn_rl_repo")

import ml_dtypes
import numpy as np

import concourse.bass as bass
import concourse.tile as tile
from concourse import bacc, mybir
from concourse.bass_utils import run_bass_kernel_spmd

B, T, C, H, D = 4, 2048, 768, 12, 64
N_CORES = 8
HPC = H // 2          # heads per core = 6
QC = T // 512         # 4 query chunks of 512
TT = T // 128         # 16 token tiles
S = 16.0              # fp8 weight prescale
F32 = mybir.dt.float32
F16 = mybir.dt.float16
BF16 = mybir.dt.bfloat16
FP8 = mybir.dt.float8e4
NPBF = ml_dtypes.bfloat16
NPF8 = ml_dtypes.float8_e4m3


def _install_ntff_hook():
    """The image's antenv lacks axon_hooks; inject it so trace=True works."""
    if "antenv.axon_hooks" in sys.modules:
        return
    try:
        import antenv
        mod = types.ModuleType("antenv.axon_hooks")
        _state = {"hook": None}
        mod.set_axon_ntff_profile_hook = lambda h: _state.__setitem__("hook", h)
        mod.get_axon_ntff_profile_hook = lambda: _state["hook"]
        sys.modules["antenv.axon_hooks"] = mod
        antenv.axon_hooks = mod
        from trn_agent_boot.trn_boot import _ntff_profile_via_ctypes
        mod.set_axon_ntff_profile_hook(
            _ntff_profile_via_ctypes("/opt/axon/libaxon_pjrt.so")
        )
    except Exception:
        pass


def _build_program(has_bv: bool, debug: bool = False, n_dev: int = N_CORES):
    nc = bacc.Bacc(
        "TRN2",
        target_bir_lowering=False,
        debug=False,
        enable_asserts=False,
        num_devices=n_dev,
    )
    xp8 = nc.dram_tensor("xp8", [128, 6 * T], FP8, kind="ExternalInput").ap()
    xb16 = nc.dram_tensor("xb16", [128, 6 * 512], BF16, kind="ExternalInput").ap()
    wqk8 = nc.dram_tensor("wqk8", [128, 4608], FP8, kind="ExternalInput").ap()
    wv8 = nc.dram_tensor("wv8", [128, 2400], FP8, kind="ExternalInput").ap()
    wv16 = nc.dram_tensor("wv16", [128, 2340], BF16, kind="ExternalInput").ap()
    wp16 = nc.dram_tensor("wp16", [128, 2304], BF16, kind="ExternalInput").ap()
    bqk = nc.dram_tensor("bqk", [128, 6], F32, kind="ExternalInput").ap()
    qmask = nc.dram_tensor("qmask", [128, 2], F32, kind="ExternalInput").ap()
    mask = nc.dram_tensor("mask", [128, 1024], BF16, kind="ExternalInput").ap()
    bv = nc.dram_tensor("bv", [1, 390], BF16, kind="ExternalInput").ap()
    ones = nc.dram_tensor("ones", [1, 128], BF16, kind="ExternalInput").ap()
    yp = nc.dram_tensor("yp", [T, C], F16, kind="ExternalOutput").ap()
    dbg = None
    if debug:
        dbg = {
            "dbg_q0": nc.dram_tensor("dbg_q0", [128, T], BF16,
                                     kind="ExternalOutput").ap(),
            "dbg_k0": nc.dram_tensor("dbg_k0", [128, T], BF16,
                                     kind="ExternalOutput").ap(),
            "dbg_v": nc.dram_tensor("dbg_v", [128, TT * 768], FP8,
                                    kind="ExternalOutput").ap(),
            "dbg_vb16": nc.dram_tensor("dbg_vb16", [128, 4 * 768], BF16,
                                       kind="ExternalOutput").ap(),
            "dbg_z": nc.dram_tensor("dbg_z", [128, 1024 * 8], F32,
                                    kind="ExternalOutput").ap(),
            "dbg_yz": nc.dram_tensor("dbg_yz", [128, 512 * 8], F32,
                                     kind="ExternalOutput").ap(),
        }

    with tile.TileContext(nc) as tc:
        _body(tc, nc, has_bv, xp8, xb16, wqk8, wv8, wv16, wp16, bqk, qmask,
              mask, bv, ones, yp, dbg)

    nc.compile()
    return nc


def _body(tc, nc, has_bv, xp8, xb16, wqk8, wv8, wv16, wp16, bqk, qmask,
          mask, bv, ones, yp, dbg=None):
    from contextlib import ExitStack
    DR = mybir.MatmulPerfMode.DoubleRow
    ADD = mybir.AluOpType.add
    MUL = mybir.AluOpType.mult
    EXP = mybir.ActivationFunctionType.Exp

    with ExitStack() as es:
        persist = es.enter_context(tc.tile_pool(name="persist", bufs=1))
        ppair = es.enter_context(tc.tile_pool(name="ppair", bufs=2, space="PSUM"))
        pyz = es.enter_context(tc.tile_pool(name="pyz", bufs=2, space="PSUM"))
        paux = es.enter_context(tc.tile_pool(name="paux", bufs=2, space="PSUM"))
        zpool = es.enter_context(tc.tile_pool(name="zpool", bufs=3))
        z16pool = es.enter_context(tc.tile_pool(name="z16pool", bufs=3))
        ypool = es.enter_context(tc.tile_pool(name="ypool", bufs=2))
        opool = es.enter_context(tc.tile_pool(name="opool", bufs=3))
        spool = es.enter_context(tc.tile_pool(name="spool", bufs=2))

        # ---- persistent tiles
        xp8s = persist.tile([128, 6 * T], FP8, tag="xp8", name="xp8s")
        xb16s = persist.tile([128, 6 * 512], BF16, tag="xb16", name="xb16s")
        wqk8s = persist.tile([128, 4608], FP8, tag="wqk8", name="wqk8s")
        wv8s = persist.tile([128, 2400], FP8, tag="wv8", name="wv8s")
        wv16s = persist.tile([128, 2340], BF16, tag="wv16", name="wv16s")
        wp16s = persist.tile([128, 2304], BF16, tag="wp16", name="wp16s")
        bqks = persist.tile([128, 6], F32, tag="bqk", name="bqks")
        qmasks = persist.tile([128, 2], F32, tag="qmask", name="qmasks")
        masks = persist.tile([128, 1024], BF16, tag="mask", name="masks")
        bvs = persist.tile([1, 390], BF16, tag="bv", name="bvs")
        oness = persist.tile([1, 128], BF16, tag="ones", name="oness")
        vbuf8 = persist.tile([128, TT * 768], FP8, tag="vbuf8", name="vbuf8")
        vb16 = persist.tile([128, 4 * 768], BF16, tag="vb16", name="vb16")
        qt = [persist.tile([128, T], BF16, tag=f"qt{h}", name=f"qt{h}")
              for h in range(HPC)]
        kt = [persist.tile([128, T], BF16, tag=f"kt{r}", name=f"kt{r}")
              for r in range(3)]

        # ---- DMA priority order. wqk8 is ft-major so b_round(0,0) only
        # gates on ft blocks 0 and 3; x window 0 on its own queue.
        nc.sync.dma_start(xp8s[:, 0:3072], xp8[:, 0:3072])
        nc.gpsimd.dma_start(wqk8s[:, 0:768], wqk8[:, 0:768])
        nc.gpsimd.dma_start(wqk8s[:, 2304:3072], wqk8[:, 2304:3072])
        nc.scalar.dma_start(bqks[:], bqk[:])
        nc.scalar.dma_start(qmasks[:], qmask[:])
        nc.scalar.dma_start(masks[:], mask[:])
        nc.scalar.dma_start(bvs[:], bv[:])
        nc.scalar.dma_start(oness[:], ones[:])
        nc.sync.dma_start(xb16s[:, 0:1536], xb16[:, 0:1536])
        nc.scalar.dma_start(xb16s[:, 1536:3072], xb16[:, 1536:3072])
        nc.gpsimd.dma_start(wv16s[:], wv16[:])
        nc.gpsimd.dma_start(wqk8s[:, 768:1536], wqk8[:, 768:1536])
        nc.gpsimd.dma_start(wqk8s[:, 3072:3840], wqk8[:, 3072:3840])
        nc.gpsimd.dma_start(wqk8s[:, 1536:2304], wqk8[:, 1536:2304])
        nc.gpsimd.dma_start(wqk8s[:, 3840:4608], wqk8[:, 3840:4608])
        nc.gpsimd.dma_start(wv8s[:], wv8[:])
        nc.gpsimd.dma_start(wp16s[:], wp16[:])

        # vbuf8 layout: block tt at tt*768, head h at +h*128: cols 0:64 = v,
        # cols 64:128 all ones -> the AV matmul replicates the softmax
        # denominator into yz rows 64:128 (free partition-broadcast).
        vb_blocks = vbuf8[:].rearrange("p (b f) -> p b f", b=TT * 6)
        nc.gpsimd.memset(vb_blocks[:, :, 64:128], 1.0)
        vb16_blocks = vb16[:].rearrange("p (b f) -> p b f", b=4 * 6)
        nc.gpsimd.memset(vb16_blocks[:, :, 64:128], 1.0)
        # z8 ring bufs zeroed once: diag-pair mask muls read stale regions
        # (finite garbage is fine, first-use NaN bit patterns are not)
        for _ in range(6):
            zi = zpool.tile([128, 1024], FP8, tag="z8", name="z8init")
            nc.gpsimd.memset(zi[:], 0.0)

        # x^T stored q4-major: [128, q4(4), chunk(6), 512]; each a_chunk DMA
        # is one contiguous 3KB-per-partition slab
        xq = xp8s[:].rearrange("p (q c t) -> p q c t", q=4, c=6)

        def xpair(p, q4):
            # fp8 x^T chunk-pair view for one 512-token window: [128, 2, 512]
            return xq[:, q4, 2 * p:2 * p + 2, :]

        def wqk_pair(p, ft):
            # ft-major layout: [128, ft(6), p(3), slot(2), 128]
            base = ft * 768 + p * 256
            return wqk8s[:, base:base + 256].rearrange(
                "p (two f) -> p two f", two=2)

        def wv_pair(p):
            return wv8s[:, p * 800:(p + 1) * 800].rearrange(
                "p (two f) -> p two f", two=2)[:, :, 0:390]

        def vpair(j, h):
            return vbuf8[:, 2 * j * 768:(2 * j + 2) * 768].rearrange(
                "p (two f) -> p two f", two=2)[:, :, h * 128:(h + 1) * 128]

        def a_chunk(q4):
            nc.sync.dma_start(xp8s[:, q4 * 3072:(q4 + 1) * 3072],
                              xp8[:, q4 * 3072:(q4 + 1) * 3072])

        def b_round(q4, r):
            # q features (ft=r) and k features (ft=r+3) for heads 2r, 2r+1
            w = slice(q4 * 512, (q4 + 1) * 512)
            ptq = paux.tile([128, 512], F32, tag="aux", name="ptq")
            ptk = paux.tile([128, 512], F32, tag="aux", name="ptk")
            for p in range(3):
                rhs = xpair(p, q4)
                nc.tensor.matmul(ptq[:], wqk_pair(p, r), rhs,
                                 start=(p == 0), stop=(p == 2), perf_mode=DR)
                nc.tensor.matmul(ptk[:], wqk_pair(p, r + 3), rhs,
                                 start=(p == 0), stop=(p == 2), perf_mode=DR)
            # (psum + 16*bias) * rowmask/16 -> unscaled q with dead rows zeroed
            nc.vector.tensor_scalar(
                qt[2 * r][:, w], ptq[:], bqks[:, r:r + 1],
                qmasks[:, 0:1], ADD, MUL)
            nc.vector.tensor_scalar(
                qt[2 * r + 1][:, w], ptq[:], bqks[:, r:r + 1],
                qmasks[:, 1:2], ADD, MUL)
            nc.vector.tensor_scalar(
                kt[r][:, w], ptk[:], bqks[:, r + 3:r + 4],
                1.0 / S, ADD, MUL)

        def c_block(q4, j):
            tt = 4 * q4 + j
            pva = paux.tile([128, 512], F32, tag="aux", name="pv")
            pv = pva[:, 0:390]
            if tt < 4:
                for cc in range(6):
                    nc.tensor.matmul(
                        pv,
                        xb16s[:, cc * 512 + tt * 128:cc * 512 + (tt + 1) * 128],
                        wv16s[:, cc * 390:(cc + 1) * 390],
                        start=(cc == 0), stop=(cc == 5) and not has_bv,
                    )
            else:
                for p in range(3):
                    nc.tensor.matmul(
                        pv,
                        xq[:, q4, 2 * p:2 * p + 2, j * 128:(j + 1) * 128],
                        wv_pair(p),
                        start=(p == 0), stop=(p == 2) and not has_bv,
                        perf_mode=DR,
                    )
            return tt, pv

        def c_finish(tt, pv):
            ncols = 65 if has_bv else 64
            if has_bv:
                nc.tensor.matmul(pv, oness[:], bvs[:],
                                 start=False, stop=True)
            # unscale x16 -> vbuf8 (fp8), strided to 128-col head slots
            nc.vector.tensor_scalar_mul(
                vb_blocks[:, tt * 6:(tt + 1) * 6, 0:ncols],
                pv.rearrange("p (b f) -> p b f", b=6)[:, :, 0:ncols],
                1.0 / S)
            if tt < 4:
                nc.vector.tensor_scalar_mul(
                    vb16_blocks[:, tt * 6:(tt + 1) * 6, 0:ncols],
                    pv.rearrange("p (b f) -> p b f", b=6)[:, :, 0:ncols],
                    1.0 / S)

        def c_piece(q4, j):
            c_finish(*c_block(q4, j))

        def sc_pair(q4, h, j):
            """Scores + exp + masks for pair j of head h of query chunk q4."""
            w0 = q4 * 512
            use8 = q4 > 0
            kb0, kb1 = 2 * j, 2 * j + 1
            off0 = max(0, kb0 * 128 - w0)
            off1 = max(0, kb1 * 128 - w0)
            diag1 = kb1 * 128 >= w0
            kk = kt[h // 2]
            qq = qt[h]
            pt = ppair.tile([128, 1024], F32, tag="pair", name="ptsc")
            nc.tensor.matmul(
                pt[:, off0:512], kk[:, kb0 * 128:(kb0 + 1) * 128],
                qq[:, w0 + off0:w0 + 512], start=True, stop=True)
            nc.tensor.matmul(
                pt[:, 512 + off1:1024], kk[:, kb1 * 128:(kb1 + 1) * 128],
                qq[:, w0 + off1:w0 + 512], start=True, stop=True)
            if use8:
                zt = zpool.tile([128, 1024], FP8, tag="z8", name="z8")
            else:
                zt = z16pool.tile([128, 1024], BF16, tag="z16", name="z16")
            if not diag1:
                nc.scalar.activation(zt[:], pt[:], EXP, scale=0.125)
            else:
                # diag pairs always have off1 == off0 + 128
                nc.scalar.activation(zt[:, off0:512], pt[:, off0:512],
                                     EXP, scale=0.125)
                nc.scalar.activation(zt[:, 512 + off1:1024],
                                     pt[:, 512 + off1:1024],
                                     EXP, scale=0.125)
                # slot0 causal band: cols off0:off0+128, keep j >= p+off0
                nc.gpsimd.tensor_mul(
                    zt[:, off0:off0 + 128], zt[:, off0:off0 + 128],
                    masks[:, 512:640])
                if use8:
                    # slot1: AV-DR reads cols >= off0; zero the stale region
                    # AND apply the band in one mul over [off0, off1+128)
                    nc.gpsimd.tensor_mul(
                        zt[:, 512 + off0:512 + off1 + 128],
                        zt[:, 512 + off0:512 + off1 + 128],
                        masks[:, 384:640])
                else:
                    nc.gpsimd.tensor_mul(
                        zt[:, 512 + off1:512 + off1 + 128],
                        zt[:, 512 + off1:512 + off1 + 128],
                        masks[:, 512:640])
            return (q4, h, j, zt, off0, off1)

        def av_pair(st, yz):
            q4, h, j, zt, off0, off1 = st
            npairs = 2 * q4 + 2
            kb0, kb1 = 2 * j, 2 * j + 1
            if q4 > 0:
                nc.tensor.matmul(
                    yz[:, off0:512],
                    vpair(j, h),
                    zt[:].rearrange("p (two n) -> p two n", two=2)
                    [:, :, off0:512],
                    start=(j == 0), stop=(j == npairs - 1), perf_mode=DR)
            else:
                nc.tensor.matmul(
                    yz[:, off0:512],
                    vb16[:, kb0 * 768 + h * 128:kb0 * 768 + (h + 1) * 128],
                    zt[:, off0:512],
                    start=(j == 0), stop=False)
                nc.tensor.matmul(
                    yz[:, off1:512],
                    vb16[:, kb1 * 768 + h * 128:kb1 * 768 + (h + 1) * 128],
                    zt[:, 512 + off1:1024],
                    start=False, stop=(j == npairs - 1))

        def norm_head(h, yz, yts):
            # yz rows 64:128 all hold the denominator (ones-cols trick)
            dcp = spool.tile([64, 512], F32, tag="dcp", name="dcp")
            nc.vector.tensor_copy(dcp[:], yz[64:128, :])
            rc = spool.tile([64, 512], F32, tag="rc", name="rc")
            nc.vector.reciprocal_approx_fast(rc[:], dcp[:])
            nc.vector.tensor_mul(
                yts[h // 2][(h % 2) * 64:(h % 2) * 64 + 64, :],
                yz[0:64, :], rc[:])

        def make_proj_halves(pq4, pyts, qt_i, pool, tag):
            state = {}

            def do_half(half):
                pp = pool.tile([128, 512], F32, tag=tag, name="pp")
                for hdc in range(3):
                    nc.tensor.matmul(
                        pp[:, 0:384],
                        pyts[hdc][:, qt_i * 128:(qt_i + 1) * 128],
                        wp16s[:, hdc * 768 + half * 384:
                              hdc * 768 + (half + 1) * 384],
                        start=(hdc == 0), stop=(hdc == 2))
                nc.vector.tensor_copy(
                    state["ot"][:, half * 384:(half + 1) * 384], pp[:, 0:384])

            def half0():
                state["ot"] = opool.tile([128, C], F16, tag="ot", name="ot")
                do_half(0)

            def half1():
                do_half(1)
                row = (pq4 * 4 + qt_i) * 128
                nc.sync.dma_start(yp[row:row + 128, :], state["ot"][:])

            return half0, half1

        # ---- braided pipeline: a flat per-pair stream with the AV matmul
        # lagging TWO pairs behind its scores (the PE queue is in-order, so
        # an AV waiting on exp/mask would otherwise block the whole stream);
        # QKV rounds / V blocks (next q4) and proj halves (prev q4)
        # interleave at head boundaries as filler PE work.
        from collections import deque
        b_round(0, 0)
        c_piece(0, 0)
        c_piece(0, 1)
        pending = None
        for q4 in range(QC):
            if q4 + 1 < QC:
                a_chunk(q4 + 1)
            yts = [ypool.tile([128, 512], BF16, tag=f"yt{i}", name=f"yt{i}")
                   for i in range(3)]
            npairs = 2 * q4 + 2
            pairs = [(h, j) for h in range(HPC) for j in range(npairs)]
            nextf = []
            if pending is not None:
                for qt_i in range(4):
                    h0, h1 = make_proj_halves(*pending, qt_i, paux, "aux")
                    nextf.append(h0)
                    nextf.append(h1)
            if q4 + 1 < QC:
                for jj in range(4):
                    nextf.append(lambda jj=jj: c_piece(q4 + 1, jj))
                for r in range(3):
                    nextf.append(lambda r=r: b_round(q4 + 1, r))
            fill_at = {}
            if q4 == 0:
                fill_at = {1: [lambda: c_piece(0, 2)],
                           2: [lambda: c_piece(0, 3)],
                           4: [lambda: b_round(0, 1)],
                           6: [lambda: b_round(0, 2)]}
                for i, f in enumerate(nextf):
                    fill_at.setdefault(8 if i < 3 else 10, []).append(f)
            else:
                for i, f in enumerate(nextf):
                    fill_at.setdefault((i % HPC) * npairs, []).append(f)
            pend = deque()
            yzs = {}

            def flush_one():
                st = pend.popleft()
                ph = st[1]
                av_pair(st, yzs[ph])
                if st[2] == npairs - 1:
                    norm_head(ph, yzs[ph], yts)

            for idx, (h, j) in enumerate(pairs):
                for f in fill_at.get(idx, []):
                    f()
                if h not in yzs:
                    yzs[h] = pyz.tile([128, 512], F32, tag="yz",
                                      name=f"yz{h}")
                pend.append(sc_pair(q4, h, j))
                if len(pend) > 2:
                    flush_one()
            while pend:
                flush_one()
            pending = (q4, yts)
        # tail: no pairs left to hide behind; the two aux banks ping-pong
        for qt_i in range(4):
            h0, h1 = make_proj_halves(*pending, qt_i, paux, "aux")
            h0()
            h1()
        if dbg is not None:
            nc.sync.dma_start(dbg["dbg_q0"][:], qt[0][:])
            nc.sync.dma_start(dbg["dbg_k0"][:], kt[0][:])
            nc.sync.dma_start(dbg["dbg_v"][:], vbuf8[:])
            nc.sync.dma_start(dbg["dbg_vb16"][:], vb16[:])


_PROGRAMS = {}


def _get_program(has_bv):
    if has_bv not in _PROGRAMS:
        _PROGRAMS[has_bv] = _build_program(has_bv)
    return _PROGRAMS[has_bv]


def _pack_rows(a, npair):
    """[rows, f] -> [128, npair, 2, f] -> [128, npair*2*f] paired row chunks."""
    rows, f = a.shape
    return np.ascontiguousarray(
        a.reshape(npair, 2, 128, f).transpose(2, 0, 1, 3).reshape(128, -1))


def _pad_wv(wv):
    out = np.zeros((C, HPC * 65), dtype=np.float32)
    for h in range(HPC):
        out[:, h * 65:h * 65 + D] = wv[:, h * D:(h + 1) * D]
    return out


def kernel(x, W_attn, b_attn, W_proj, b_proj):
    x = np.ascontiguousarray(x, dtype=np.float32)
    W_attn = np.ascontiguousarray(W_attn, dtype=np.float32)
    b_attn = np.ascontiguousarray(b_attn, dtype=np.float32)
    W_proj = np.ascontiguousarray(W_proj, dtype=np.float32)
    b_proj = np.ascontiguousarray(b_proj, dtype=np.float32)

    has_bv = bool(np.any(b_attn[2 * C:]))
    nc = _get_program(has_bv)

    # mask[p, c] = 1 iff c >= p + 512: slicing [:, 512:640] gives the
    # causal triu band; [:, 384:640] additionally zeroes 128 garbage cols
    ci = np.arange(1024)[None, :]
    pi = np.arange(128)[:, None]
    mask_const = (ci >= pi + 512).astype(np.float32).astype(NPBF)
    qmask_const = np.zeros((128, 2), np.float32)
    qmask_const[0:64, 0] = 1.0 / S
    qmask_const[64:128, 1] = 1.0 / S
    ones_const = np.ones((1, 128), dtype=NPBF)

    in_maps = []
    for core in range(N_CORES):
        b, g = core // 2, core % 2
        qcols = slice(384 * g, 384 * (g + 1))
        kcols = slice(768 + 384 * g, 768 + 384 * (g + 1))
        vcols = slice(1536 + 384 * g, 1536 + 384 * (g + 1))

        xt = np.ascontiguousarray(x[b].T)                      # [768, 2048]
        # q4-major: xp8[p, q4*3072 + cc*512 + t'] = x^T[cc*128+p, q4*512+t']
        xp8_np = xt.reshape(6, 128, 4, 512).transpose(1, 2, 0, 3) \
            .reshape(128, 6 * T)
        xb16_np = xt[:, 0:512].reshape(6, 128, 512).transpose(1, 0, 2) \
            .reshape(128, 6 * 512)

        wcat = np.concatenate([W_attn[:, qcols], W_attn[:, kcols]], axis=1)
        # ft-major: wqk8[pr, ft*768 + p*256 + i*128 + f]
        #         = (wcat*S)[(2p+i)*128 + pr, ft*128 + f]
        wqk8_np = (wcat * S).reshape(3, 2, 128, 6, 128) \
            .transpose(2, 3, 0, 1, 4).reshape(128, 4608)

        wvp = _pad_wv(W_attn[:, vcols]) * S                    # [768, 390]
        wv8_pairs = wvp.reshape(3, 2, 128, 390).transpose(2, 0, 1, 3)
        wv8_np = np.zeros((128, 3, 2, 400), np.float32)
        wv8_np[:, :, :, 0:390] = wv8_pairs
        wv8_np = wv8_np.reshape(128, 2400)
        wv16_np = wvp.reshape(6, 128, 390).transpose(1, 0, 2).reshape(128, -1)

        wp16_np = W_proj[384 * g:384 * (g + 1), :] \
            .reshape(3, 128, C).transpose(1, 0, 2).reshape(128, -1)

        bqk_np = np.concatenate([b_attn[qcols], b_attn[kcols]]) \
            .reshape(6, 128).T * S

        bv_np = np.zeros((1, 390), np.float32)
        bvg = b_attn[vcols]
        for h in range(HPC):
            bv_np[0, h * 65:h * 65 + D] = bvg[h * D:(h + 1) * D] * S
            bv_np[0, h * 65 + D] = S

        in_maps.append({
            "xp8": xp8_np.astype(NPF8),
            "xb16": xb16_np.astype(NPBF),
            "wqk8": wqk8_np.astype(NPF8),
            "wv8": wv8_np.astype(NPF8),
            "wv16": wv16_np.astype(NPBF),
            "wp16": np.ascontiguousarray(wp16_np).astype(NPBF),
            "bqk": np.ascontiguousarray(bqk_np, dtype=np.float32),
            "qmask": qmask_const,
            "mask": mask_const,
            "bv": bv_np.astype(NPBF),
            "ones": ones_const,
        })

    trace = bool(int(os.environ.get("KBENCH_TRACE", "0")))
    if trace:
        _install_ntff_hook()
    res = run_bass_kernel_spmd(
        nc, in_maps, list(range(N_CORES)), trace=trace,
    )
    kernel.last_exec_time_ns = res.exec_time_ns

    out = np.empty((B, T, C), dtype=np.float32)
    for b in range(B):
        out[b] = (res.results[2 * b]["yp"].astype(np.float32)
                  + res.results[2 * b + 1]["yp"].astype(np.float32)
                  + b_proj)
    return out
